# revision 1
# baseline (speedup 1.0000x reference)
"""nn_LocalSpatialEncoding Trainium2 kernel (Bass/Tile, 8 NeuronCores).

Takes the FULL inputs of the reference problem (B=4, N=16384, K=16, D=16),
shards over (batch, point-range) across 8 cores, runs one SPMD Bass kernel
(gather via gpsimd ap_gather, conv as an algebraic split of the 1x1 conv,
BN batch stats via on-device AllReduce), and reassembles the full output.

Decomposition of the conv (channel c, point n, neighbor k):
    x[c,n,k] = A'[c]@[coords[n],1] + C[c]@coords[idx[n,k]] + e[c]*dist[n,k]
    A' = w[:,0:3]+w[:,6:9] (+bias col), C = w[:,3:6]-w[:,6:9], e = w[:,9]
so the only data-dependent movement is a gather of the 16-channel table
Q = C@coords^T, done on-chip (table replicated per 16-partition slab).
"""
import numpy as np
from contextlib import ExitStack

import concourse.bacc as bacc
import concourse.tile as tile
from concourse import mybir
from concourse.bass_utils import run_bass_kernel_spmd

F32 = mybir.dt.float32
I16 = mybir.dt.int16
EPS = 1e-6
K = 16
D = 16
NSLAB = 8

# full-problem config (hardcoded)
B = 4
N = 16384
NL = 8192            # points per core
N_CORES = 8
CH = 1024            # x columns per streamed chunk
Mslab = NL * K // NSLAB
NCH = Mslab // CH
PL = NL // NSLAB
GU = max(1, N // 2048)
W = N // GU
COUNT = B * N * K

IN_NAMES = ['coordsT', 'coordsAlt', 'idxw', 'distf', 'feat',
            'lhsT_C', 'lhsT_P', 'e_col', 'gb16', 'red16', 'rep128']


def _prep_params(conv_w, conv_b, gamma, beta):
    A = np.concatenate(
        [conv_w[:, 0:3] + conv_w[:, 6:9], conv_b[:, None]], axis=1
    ).astype(np.float32)
    C = (conv_w[:, 3:6] - conv_w[:, 6:9]).astype(np.float32)
    e = conv_w[:, 9].astype(np.float32)

    lhsT_C = np.zeros((3, 128), np.float32)
    lhsT_P = np.zeros((4 * NSLAB, 128), np.float32)
    for a in range(NSLAB):
        lhsT_C[:, 16 * a:16 * a + 16] = C.T
        lhsT_P[4 * a:4 * a + 4, 16 * a:16 * a + 16] = A.T
    e_col = np.tile(e, NSLAB)[:, None].astype(np.float32)
    gb16 = np.stack([gamma, beta], axis=1).astype(np.float32)
    red16 = np.zeros((128, 16), np.float32)
    rep128 = np.zeros((16, 128), np.float32)
    eye = np.eye(16, dtype=np.float32)
    for a in range(NSLAB):
        red16[16 * a:16 * a + 16, :] = eye
        rep128[:, 16 * a:16 * a + 16] = eye
    return dict(lhsT_C=lhsT_C, lhsT_P=lhsT_P, e_col=e_col, gb16=gb16,
                red16=red16, rep128=rep128)


def _prep_core(coords_b, idx_s, dist_s, feat_s, params, n0):
    coordsT = np.ascontiguousarray(coords_b.T.astype(np.float32))
    coordsAlt = np.zeros((4 * NSLAB, PL), np.float32)
    for a in range(NSLAB):
        coordsAlt[4 * a:4 * a + 3, :] = \
            coords_b[n0 + a * PL:n0 + (a + 1) * PL, :].T
        coordsAlt[4 * a + 3, :] = 1.0

    idx_flat = idx_s.reshape(NSLAB, Mslab)
    dist_flat = np.ascontiguousarray(
        dist_s.reshape(NSLAB, Mslab).astype(np.float32))
    idxw = np.zeros((128, Mslab // 16), np.int16)
    for p in range(16):
        idxw[p::16, :] = idx_flat[:, p::16]

    feat128 = np.zeros((128, PL), np.float32)
    for a in range(NSLAB):
        feat128[16 * a:16 * a + 16, :] = feat_s[:, a * PL:(a + 1) * PL]

    d = dict(coordsT=coordsT, coordsAlt=coordsAlt, idxw=idxw,
             distf=dist_flat, feat=feat128)
    d.update(params)
    return d


def shard_inputs(coords, features, idx, dist, conv_w, conv_b, gamma, beta):
    params = _prep_params(conv_w, conv_b, gamma, beta)
    per_core = []
    for c in range(N_CORES):
        b, h = c // 2, c % 2
        sl = slice(h * NL, (h + 1) * NL)
        per_core.append(_prep_core(
            coords[b], idx[b][sl], dist[b][sl], features[b, :, sl, 0],
            params, h * NL))
    return per_core


def build_kernel(tc, outs, ins, use_collective=True, repeat=1):
    for _r in range(repeat):
        _build_once(tc, outs, ins, use_collective, f"r{_r}" if repeat > 1
                    else "")


def _build_once(tc, outs, ins, use_collective, pfx):
    nc = tc.nc
    t = dict(zip(IN_NAMES, ins))
    out_d = outs[0]

    ctx = ExitStack()
    sb = ctx.enter_context(tc.tile_pool(name=pfx + "fixed", bufs=1))
    ps = ctx.enter_context(tc.tile_pool(name=pfx + "psum", bufs=2, space="PSUM"))
    dram = ctx.enter_context(tc.tile_pool(name=pfx + "dram", bufs=1, space="DRAM"))
    ld_ctx = ExitStack()
    ld = ld_ctx.enter_context(tc.tile_pool(name=pfx + "qbuild", bufs=1))

    # ---------- param / table loads ----------
    lhsT_C_t = sb.tile([3, 128], F32)
    nc.sync.dma_start(out=lhsT_C_t[:], in_=t['lhsT_C'][:])
    # ---------- Q table (replicated across slabs) ----------
    qrep = sb.tile([128, N], F32)
    per_u = W // 512
    for u in range(GU):
        cT_t = ld.tile([3, W], F32, tag="cT", bufs=2, name=f"{pfx}cT{u}")
        nc.sync.dma_start(out=cT_t[:], in_=t['coordsT'][:][:, u * W:(u + 1) * W])
        for s in range(per_u):
            tq = u * per_u + s
            qp = ps.tile([128, 512], F32, tag="qp", bufs=4, name=f"{pfx}qp{tq}")
            nc.tensor.matmul(out=qp[:], lhsT=lhsT_C_t[:],
                             rhs=cT_t[:, s * 512:s * 512 + 512],
                             start=True, stop=True)
            nc.vector.tensor_copy(out=qrep[:, tq * 512:tq * 512 + 512],
                                  in_=qp[:])

    coordsAlt_t = ld.tile([4 * NSLAB, PL], F32)
    nc.sync.dma_start(out=coordsAlt_t[:], in_=t['coordsAlt'][:])
    lhsT_P_t = sb.tile([4 * NSLAB, 128], F32)
    nc.sync.dma_start(out=lhsT_P_t[:], in_=t['lhsT_P'][:])
    idx_t = sb.tile([128, Mslab // 16], I16)
    nc.sync.dma_start(out=idx_t[:], in_=t['idxw'][:])
    feat_t = sb.tile([128, PL], F32)
    nc.sync.dma_start(out=feat_t[:], in_=t['feat'][:])
    e_t = sb.tile([128, 1], F32)
    nc.sync.dma_start(out=e_t[:], in_=t['e_col'][:])
    gb_t = sb.tile([16, 2], F32)
    nc.sync.dma_start(out=gb_t[:], in_=t['gb16'][:])
    red_t = sb.tile([128, 16], F32)
    nc.sync.dma_start(out=red_t[:], in_=t['red16'][:])
    rep_t = sb.tile([16, 128], F32)
    nc.sync.dma_start(out=rep_t[:], in_=t['rep128'][:])

    x_view = out_d[:][0:16, :, :].rearrange("c (a m) k -> a c (m k)", a=NSLAB)
    f_view = out_d[:][16:32, :, :].rearrange("c (a m) k -> a c (m k)", a=NSLAB)

    # ---------- P table (per-point term + conv bias) ----------
    p_sb = sb.tile([128, PL], F32)
    for m0 in range(0, PL, 512):
        mw = min(512, PL - m0)
        pp = ps.tile([128, mw], F32, tag="pp", name=f"{pfx}pp{m0}")
        nc.tensor.matmul(out=pp[:], lhsT=lhsT_P_t[:],
                         rhs=coordsAlt_t[:, m0:m0 + mw], start=True, stop=True)
        nc.vector.tensor_copy(out=p_sb[:, m0:m0 + mw], in_=pp[:])
    ld_ctx.close()
    st = ctx.enter_context(tc.tile_pool(name=pfx + "stream", bufs=2))

    # ---------- feats half (independent stream) ----------
    CHI = CH // 16
    CHM = CH // 16
    for j in range(NCH):
        c0 = j * CH
        m0 = j * CHM
        f16 = st.tile([128, CH], F32, tag="f16", name=f"{pfx}f16{j}")
        f_bc = (feat_t[:, m0:m0 + CHM].unsqueeze(2)
                .broadcast_to((128, CHM, 16)))
        nc.scalar.activation(
            out=f16[:].rearrange("p (m k) -> p m k", k=16), in_=f_bc,
            func=mybir.ActivationFunctionType.Copy)
        nc.sync.dma_start(out=f_view[:, :, c0:c0 + CH], in_=f16[:])

    # ---------- streaming phase 1: x = P + Q[idx] + e*dist; stats ----------
    X = sb.tile([128, Mslab], F32)
    s1col = sb.tile([128, NCH], F32)
    s2col = sb.tile([128, NCH], F32)

    for j in range(NCH):
        c0 = j * CH
        qg = st.tile([128, CH], F32, tag="qg", name=f"{pfx}qg{j}")
        nc.gpsimd.ap_gather(
            out_ap=qg[:].unsqueeze(2), in_ap=qrep[:].unsqueeze(2),
            idxs_ap=idx_t[:, j * CHI:(j + 1) * CHI],
            channels=128, num_elems=N, d=1, num_idxs=CH)

        dist_tile = st.tile([128, CH], F32, tag="dist", name=f"{pfx}dist{j}")
        dist_ap = (t['distf'][:][:, c0:c0 + CH]
                   .unsqueeze(1).broadcast_to((NSLAB, 16, CH)))
        nc.sync.dma_start(out=dist_tile[:], in_=dist_ap)

        tmp = st.tile([128, CH], F32, tag="tmp", name=f"{pfx}tmp{j}")
        nc.vector.scalar_tensor_tensor(
            out=tmp[:], in0=dist_tile[:], scalar=e_t[:, 0:1], in1=qg[:],
            op0=mybir.AluOpType.mult, op1=mybir.AluOpType.add)

        m0 = j * CHM
        p_bc = (p_sb[:, m0:m0 + CHM].unsqueeze(2)
                .broadcast_to((128, CHM, 16)))
        nc.vector.scalar_tensor_tensor(
            out=X[:, c0:c0 + CH].rearrange("p (m k) -> p m k", k=16),
            in0=tmp[:].rearrange("p (m k) -> p m k", k=16),
            scalar=1.0, in1=p_bc,
            op0=mybir.AluOpType.mult, op1=mybir.AluOpType.add,
            accum_out=s1col[:, j:j + 1])

        sq = st.tile([128, CH], F32, tag="scr", name=f"{pfx}sq{j}")
        nc.scalar.activation(
            out=sq[:], in_=X[:, c0:c0 + CH],
            func=mybir.ActivationFunctionType.Square,
            accum_out=s2col[:, j:j + 1])

    # ---------- stats: reduce, all-reduce, scale/bias ----------
    stats2 = sb.tile([128, 2], F32)
    nc.vector.tensor_reduce(out=stats2[:, 0:1], in_=s1col[:],
                            axis=mybir.AxisListType.X, op=mybir.AluOpType.add)
    nc.vector.tensor_reduce(out=stats2[:, 1:2], in_=s2col[:],
                            axis=mybir.AxisListType.X, op=mybir.AluOpType.add)
    ps16 = ps.tile([16, 2], F32, tag="ps16", bufs=1)
    nc.tensor.matmul(out=ps16[:], lhsT=red_t[:], rhs=stats2[:],
                     start=True, stop=True)
    sb16 = sb.tile([16, 2], F32)
    nc.vector.tensor_copy(out=sb16[:], in_=ps16[:])

    cc_in = dram.tile([16, 2], F32)
    cc_out = dram.tile([16, 2], F32)
    nc.gpsimd.dma_start(out=cc_in[:], in_=sb16[:])
    if use_collective:
        nc.gpsimd.collective_compute(
            "AllReduce", mybir.AluOpType.add,
            replica_groups=[list(range(N_CORES))],
            ins=[cc_in.opt()], outs=[cc_out.opt()])
    else:
        nc.gpsimd.dma_start(out=cc_out[:], in_=cc_in[:])
    g16 = sb.tile([16, 2], F32)
    nc.gpsimd.dma_start(out=g16[:], in_=cc_out[:])

    ms = sb.tile([16, 2], F32)
    nc.vector.tensor_scalar(out=ms[:], in0=g16[:], scalar1=1.0 / COUNT,
                            scalar2=None, op0=mybir.AluOpType.mult)
    var16 = sb.tile([16, 1], F32)
    nc.vector.tensor_tensor(out=var16[:], in0=ms[:, 0:1], in1=ms[:, 0:1],
                            op=mybir.AluOpType.mult)
    nc.vector.tensor_tensor(out=var16[:], in0=ms[:, 1:2], in1=var16[:],
                            op=mybir.AluOpType.subtract)
    eps_t = sb.tile([16, 1], F32)
    nc.vector.memset(eps_t[:], EPS)
    std16 = sb.tile([16, 1], F32)
    nc.scalar.activation(out=std16[:], in_=var16[:],
                         func=mybir.ActivationFunctionType.Sqrt,
                         bias=eps_t[:, 0:1])
    rstd16 = sb.tile([16, 1], F32)
    nc.vector.reciprocal(out=rstd16[:], in_=std16[:])
    sc16 = sb.tile([16, 2], F32)
    nc.vector.tensor_tensor(out=sc16[:, 0:1], in0=gb_t[:, 0:1], in1=rstd16[:],
                            op=mybir.AluOpType.mult)
    tmu = sb.tile([16, 1], F32)
    nc.vector.tensor_tensor(out=tmu[:], in0=ms[:, 0:1], in1=sc16[:, 0:1],
                            op=mybir.AluOpType.mult)
    nc.vector.tensor_tensor(out=sc16[:, 1:2], in0=gb_t[:, 1:2], in1=tmu[:],
                            op=mybir.AluOpType.subtract)
    psr = ps.tile([128, 2], F32, tag="psr", bufs=1)
    nc.tensor.matmul(out=psr[:], lhsT=rep_t[:], rhs=sc16[:],
                     start=True, stop=True)
    sb_col = sb.tile([128, 2], F32)
    nc.vector.tensor_copy(out=sb_col[:], in_=psr[:])

    # ---------- finalize: relu(x*s0+s1) -> DRAM ----------
    for j in range(NCH):
        c0 = j * CH
        ox = st.tile([128, CH], F32, tag="scr", name=f"{pfx}ox{j}")
        nc.scalar.activation(
            out=ox[:], in_=X[:, c0:c0 + CH],
            func=mybir.ActivationFunctionType.Relu,
            scale=sb_col[:, 0:1], bias=sb_col[:, 1:2])
        nc.sync.dma_start(out=x_view[:, :, c0:c0 + CH], in_=ox[:])

    ctx.close()


_COMPILED = None


def _get_compiled():
    global _COMPILED
    if _COMPILED is not None:
        return _COMPILED
    nc = bacc.Bacc("TRN2", target_bir_lowering=False, debug=False,
                   num_devices=N_CORES)
    shapes = dict(
        coordsT=(3, N), coordsAlt=(4 * NSLAB, PL), idxw=(128, Mslab // 16),
        distf=(NSLAB, Mslab), feat=(128, PL), lhsT_C=(3, 128),
        lhsT_P=(4 * NSLAB, 128), e_col=(128, 1), gb16=(16, 2),
        red16=(128, 16), rep128=(16, 128))
    dtypes = dict(idxw=I16)
    in_aps = []
    for name in IN_NAMES:
        in_aps.append(nc.dram_tensor(
            name, shapes[name], dtypes.get(name, F32),
            kind="ExternalInput").ap())
    out_ap = nc.dram_tensor("out", (2 * D, NL, K), F32,
                            kind="ExternalOutput").ap()
    with tile.TileContext(nc) as tc:
        build_kernel(tc, [out_ap], in_aps)
    nc.compile()
    _COMPILED = nc
    return nc


def run_sharded(per_core, trace=False, **kw):
    nc = _get_compiled()
    in_maps = [{k: pc[k] for k in IN_NAMES} for pc in per_core]
    return run_bass_kernel_spmd(nc, in_maps, list(range(N_CORES)),
                                trace=trace, **kw)


def kernel(coords, features, idx, dist, conv_w, conv_b, bn_gamma, bn_beta):
    coords = np.asarray(coords, dtype=np.float32)
    features = np.asarray(features, dtype=np.float32)
    idx = np.asarray(idx)
    dist = np.asarray(dist, dtype=np.float32)
    conv_w = np.asarray(conv_w, dtype=np.float32)
    conv_b = np.asarray(conv_b, dtype=np.float32)
    bn_gamma = np.asarray(bn_gamma, dtype=np.float32)
    bn_beta = np.asarray(bn_beta, dtype=np.float32)

    per_core = shard_inputs(coords, features, idx, dist, conv_w, conv_b,
                            bn_gamma, bn_beta)
    res = run_sharded(per_core)
    out = np.empty((B, 2 * D, N, K), np.float32)
    for c in range(N_CORES):
        b, h = c // 2, c % 2
        out[b, :, h * NL:(h + 1) * NL, :] = res.results[c]['out']
    return out

